# revision 16
# baseline (speedup 1.0000x reference)
"""Multi-head causal attention (B=2, S=2048, D=4096, H=32, hd=128) on 8 trn2 cores.

Sharding: DP over batch (2) x TP over heads (4 groups of 8 heads).
Core c: batch b = c//4, head-group tp = c%4.
Each core computes a partial output [2048, 4096] (wo row-sharded); host sums
the 4 partials per batch.

v2 design notes:
- All shipped data (inputs + output partials) is float16, packed into ONE
  blob tensor per core: per-execute staging in the axon runtime scales with
  both tensor count and bytes, so 13 tensors/146MB -> 2 tensors/68MB.
- On-chip: projections run fp16 x fp16 -> f32 PSUM (full PE rate), RoPE in
  f32 on DVE, q/k/v spilled to device DRAM as fp16. Attention matmuls are
  fp16 (scores PSUM f32, exp on Activation engine writes fp16 at 2x rate).
- Causal mask is multiplicative 0/1 applied AFTER exp, only on the 4
  diagonal block patterns (one [128,512] fp16 pattern per j-4s offset), and
  scores/exp/PV/Dn work on diagonal blocks is restricted to the visible
  >=256-wide query sub-range.
- Softmax denominator via ones-column matmuls accumulated in PSUM (the ones
  vector is column 511 of mask pattern 0).
- Attention inner loop is software-pipelined (scores/exp staged 2 ahead of
  the PV matmul) so the PE never waits on the Activation engine.
- Second x-strip is prefetched during the first strip's compute.
"""

import sys
sys.path.insert(0, '/opt/trn_rl_repo')
sys.path.insert(0, '/opt/trn_rl_repo/concourse')

import numpy as np
from contextlib import ExitStack

S = 2048
D = 4096
HD = 128
FSH = 1024            # features per core (8 heads)
NHL = 8               # heads per core
KT = D // 128         # 32 k-tiles for projections
TSTRIPS = S // 512    # 4 tq strips
NKT = S // 128        # 16 tk tiles
NEG_THRESH = -1.0e8

# blob layout: (name, rows, cols), fp16, offsets 32-element aligned
_LAYOUT = [
    ("xt", D, S),
    ("wqt", D, FSH),
    ("wkt", D, FSH),
    ("wvt", D, FSH),
    ("wot", FSH, D),
    ("cosw", 64, S),
    ("sinw", 64, S),
    ("nsinw", 64, S),
    ("mask4", 4 * 128, 512),
    ("id128", 128, 128),
]


def _offsets():
    offs, off = {}, 0
    for name, r, c in _LAYOUT:
        offs[name] = off
        off += r * c
        off = (off + 31) & ~31
    return offs, off


_OFFS, _BLOB_N = _offsets()

_cache = {}


def _build(classes):
    """Build + compile the per-core Bacc program. classes[j][s] in {0:skip,1:full,2:diag}."""
    import concourse.bacc as bacc
    import concourse.mybir as mybir
    import concourse.tile as tile
    from concourse import bass_isa

    f16 = mybir.dt.float16
    f32 = mybir.dt.float32
    f32r = mybir.dt.float32r
    EXP = mybir.ActivationFunctionType.Exp
    COPY = mybir.ActivationFunctionType.Copy

    nc = bacc.Bacc("TRN2", target_bir_lowering=False, debug=False)

    blob = nc.dram_tensor("blob", [_BLOB_N], f16, kind="ExternalInput").ap()
    out_d = nc.dram_tensor("out", [S, D], f16, kind="ExternalOutput").ap()

    def view(name):
        for nm, r, c in _LAYOUT:
            if nm == name:
                o = _OFFS[name]
                return blob[o:o + r * c].rearrange("(r c) -> r c", r=r)
        raise KeyError(name)

    xt_d = view("xt")
    wqt_d = view("wqt")
    wkt_d = view("wkt")
    wvt_d = view("wvt")
    wot_d = view("wot")
    cos_d = view("cosw")
    sin_d = view("sinw")
    nsin_d = view("nsinw")
    mask_d = view("mask4")
    id_d = view("id128")

    with tile.TileContext(nc) as tc, \
         nc.allow_low_precision(reason="fp16 everywhere is within 2e-2 tolerance"):
        with tc.tile_pool(name="pdram", bufs=1, space="DRAM") as pdram, \
             tc.tile_pool(name="pconst", bufs=1) as pconst, \
             tc.tile_pool(name="p2h", bufs=3) as p2h:
            qt_d = pdram.tile([FSH, S], f16, name="qt_spill")
            kt_d = pdram.tile([FSH, S], f16, name="kt_spill")
            vt_d = pdram.tile([FSH, S], f16, name="vt_spill")
            id_sb = pconst.tile([128, 128], f16, name="id_sb")
            nc.sync.dma_start(out=id_sb, in_=id_d)

            def load_qkv(h):
                vt_h = p2h.tile([128, S], f16, name="vt_h")
                kt_h = p2h.tile([128, S], f16, name="kt_h")
                qt_h = p2h.tile([128, S], f16, name="qt_h")
                nc.sync.dma_start(out=vt_h, in_=vt_d[h * 128:(h + 1) * 128, :])
                nc.sync.dma_start(out=kt_h, in_=kt_d[h * 128:(h + 1) * 128, :])
                nc.sync.dma_start(out=qt_h, in_=qt_d[h * 128:(h + 1) * 128, :])
                return vt_h, kt_h, qt_h
            cos_sb = pconst.tile([64, S], f16, name="cos_sb")
            sin_sb = pconst.tile([64, S], f16, name="sin_sb")

            # ---------------- Phase 1: q/k/v projections (+RoPE on q,k) -------------
            with ExitStack() as st1:
                p1x = st1.enter_context(tc.tile_pool(name="p1x", bufs=KT + 16))
                p1w = st1.enter_context(tc.tile_pool(name="p1w", bufs=3))
                p1c16 = st1.enter_context(tc.tile_pool(name="p1c16", bufs=1))
                p1t = st1.enter_context(tc.tile_pool(name="p1t", bufs=6))
                p1o = st1.enter_context(tc.tile_pool(name="p1o", bufs=6))
                ps1 = st1.enter_context(tc.tile_pool(name="ps1", bufs=4, space="PSUM"))
                w_ds = [wqt_d, wkt_d, wvt_d]
                spills = [qt_d, kt_d, vt_d]

                woffs = [_OFFS["wqt"], _OFFS["wkt"], _OFFS["wvt"]]

                def load_w(proj, i):
                    # weights are packed host-side in SBUF layout [i][p][k][f]
                    # so each tile load is one fully contiguous 1MB DMA
                    wt = p1w.tile([128, KT, 128], f16, name="wt")
                    base = woffs[proj] + i * (128 * KT * 128)
                    w_ap = blob[base:base + 128 * KT * 128].rearrange(
                        "(p k f) -> p k f", p=128, k=KT)
                    nc.scalar.dma_start(out=wt, in_=w_ap)
                    return wt

                def load_x(T2, k):
                    t0 = T2 * 1024
                    xt_t = p1x.tile([128, 1024], f16, name="xk")
                    nc.sync.dma_start(
                        out=xt_t, in_=xt_d[k * 128:(k + 1) * 128, t0:t0 + 1024])
                    return xt_t

                # first weight tile before the x strip so job 0 starts promptly
                jobs = [(T2, proj, i) for T2 in range(2)
                        for proj in range(3) for i in range(NHL)]
                wt_next = load_w(jobs[0][1], jobs[0][2])
                # strip-0 x tiles + rope tables
                xk_strips = [[load_x(0, k) for k in range(KT)], [None] * KT]
                for c16_d, csb in ((cos_d, cos_sb), (sin_d, sin_sb)):
                    nc.scalar.dma_start(out=csb, in_=c16_d)
                nprefetch = 0
                for idx, (T2, proj, i) in enumerate(jobs):
                    wt = wt_next
                    if idx + 1 < len(jobs):
                        wt_next = load_w(jobs[idx + 1][1], jobs[idx + 1][2])
                    # prefetch strip-1 x tiles during the tail of strip 0; the
                    # last 16 reuse strip-0 slots (Tile inserts the WAR dep)
                    if T2 == 0 and nprefetch < KT:
                        lim = 16 if idx < 22 else KT
                        if idx >= 12:
                            for _ in range(3 if idx < 22 else 8):
                                if nprefetch < lim:
                                    xk_strips[1][nprefetch] = load_x(1, nprefetch)
                                    nprefetch += 1
                    t0 = T2 * 1024
                    xk = xk_strips[T2]
                    spill = spills[proj]
                    if idx == 41:
                        nxt_qkv = load_qkv(0)
                    for tsub in range(2):
                        ps = ps1.tile([128, 512], f32, name="ps1")
                        for k in range(KT):
                            nc.tensor.matmul(
                                ps, wt[:, k, :],
                                xk[k][:, tsub * 512:(tsub + 1) * 512],
                                start=(k == 0), stop=(k == KT - 1))
                        ot = p1o.tile([128, 512], f16, name="ot")
                        csl = slice(t0 + tsub * 512, t0 + (tsub + 1) * 512)
                        if proj < 2:  # RoPE for q, k (f16 on DVE, ACT downconvert)
                            pc_re = p1t.tile([64, 512], f16, name="pc_re")
                            pc_im = p1t.tile([64, 512], f16, name="pc_im")
                            nc.scalar.activation(pc_re, ps[0:64], COPY)
                            nc.scalar.activation(pc_im, ps[64:128], COPY)
                            m1 = p1t.tile([64, 512], f16, name="m1")
                            m2 = p1t.tile([64, 512], f16, name="m2")
                            nc.vector.tensor_mul(m1, pc_re, cos_sb[:, csl])
                            nc.vector.tensor_mul(m2, pc_im, sin_sb[:, csl])
                            nc.vector.tensor_sub(ot[0:64], m1, m2)
                            m3 = p1t.tile([64, 512], f16, name="m1")
                            m4 = p1t.tile([64, 512], f16, name="m2")
                            nc.vector.tensor_mul(m3, pc_re, sin_sb[:, csl])
                            nc.vector.tensor_mul(m4, pc_im, cos_sb[:, csl])
                            nc.vector.tensor_add(ot[64:128], m3, m4)
                        else:
                            nc.scalar.activation(ot, ps, COPY)
                        nc.sync.dma_start(
                            out=spill[i * 128:(i + 1) * 128,
                                      t0 + tsub * 512:t0 + (tsub + 1) * 512],
                            in_=ot)

            # ---------------- Phase 2: attention per head ----------------------------
            with ExitStack() as st0:
              patt = st0.enter_context(tc.tile_pool(name="patt", bufs=1))
              p3w = st0.enter_context(tc.tile_pool(name="p3w", bufs=2))
              att8 = patt.tile([128, NHL, S], f16, name="att8")

              def load_w3(c):
                  wt = p3w.tile([128, NHL, 512], f16, name="w3")
                  base = _OFFS["wot"] + c * (128 * NHL * 512)
                  w_ap = blob[base:base + 128 * NHL * 512].rearrange(
                      "(p k f) -> p k f", p=128, k=NHL)
                  nc.sync.dma_start(out=wt, in_=w_ap)
                  return wt

              wt_next3 = load_w3(0)
              with ExitStack() as st2:
                  p2v = st2.enter_context(tc.tile_pool(name="p2v", bufs=2 * NKT + 1))
                  p2e = st2.enter_context(tc.tile_pool(name="p2e", bufs=7))
                  p2m = st2.enter_context(tc.tile_pool(name="p2m", bufs=7))
                  p2r2 = st2.enter_context(tc.tile_pool(name="p2r2", bufs=2))
                  p2o = st2.enter_context(tc.tile_pool(name="p2o", bufs=2))
                  p2msk = st2.enter_context(tc.tile_pool(name="p2msk", bufs=1))
                  ps2s = st2.enter_context(tc.tile_pool(name="ps2s", bufs=3, space="PSUM"))
                  ps2a = st2.enter_context(tc.tile_pool(name="ps2a", bufs=2, space="PSUM"))
                  ps2d = st2.enter_context(tc.tile_pool(name="ps2d", bufs=1, space="PSUM"))
                  ps2t = st2.enter_context(tc.tile_pool(name="ps2t", bufs=2, space="PSUM"))
                  mask_sb = p2msk.tile([128, 4, 512], f16, name="mask_sb")
                  nc.scalar.dma_start(
                      out=mask_sb,
                      in_=mask_d.rearrange("(four p) f -> p four f", p=128))
                  # mask pattern 0, column 511 is all-ones: the Dn reduction vector
                  ones_k = mask_sb[:, 0, 511:512]

                  for h in range(NHL):
                      vt_h, kt_h, qt_h = nxt_qkv
                      v_sb = []
                      for j in range(NKT):
                          tps = ps2t.tile([128, 128], f16, name="tp")
                          nc.tensor.transpose(tps, vt_h[:, j * 128:(j + 1) * 128], id_sb)
                          vj = p2v.tile([128, 128], f16, name="vj")
                          nc.vector.tensor_copy(vj, tps)
                          v_sb.append(vj)
                      if h + 1 < NHL:
                          nxt_qkv = load_qkv(h + 1)
                      for s in range(TSTRIPS):
                          act = [j for j in range(NKT) if classes[j][s] != 0]
                          n = len(act)
                          A = ps2a.tile([128, 512], f32, name="A")
                          Dn = ps2d.tile([1, 512], f32, name="Dn")
                          Es = {}

                          # visible query sub-range per block: diagonal block
                          # p = j-4s only sees q >= 128p, so restrict the
                          # scores/exp/PV/Dn work to a >=256-wide right slice
                          # (min 256 keeps fp16 matmuls at full PE rate).
                          def qrange(j, s=s):
                              if classes[j][s] != 2:
                                  return 0, 512
                              w = max(256, 512 - 128 * (j - 4 * s))
                              return 512 - w, w

                          def emit_front(ii, act=act, s=s, Es=Es):
                              j = act[ii]
                              qlo, w = qrange(j)
                              sps = ps2s.tile([128, 512], f32, name="sps")
                              nc.tensor.matmul(
                                  sps[:, 0:w], kt_h[:, j * 128:(j + 1) * 128],
                                  qt_h[:, s * 512 + qlo:s * 512 + qlo + w],
                                  start=True, stop=True)
                              E = p2e.tile([128, 512], f16, name="E")
                              nc.scalar.activation(E[:, 0:w], sps[:, 0:w], EXP)
                              if classes[j][s] == 2:
                                  Em = p2m.tile([128, 512], f16, name="Em")
                                  nc.vector.tensor_mul(
                                      Em[:, 0:w], E[:, 0:w],
                                      mask_sb[:, j - 4 * s, qlo:qlo + w])
                                  E = Em
                              Es[ii] = E

                          def emit_back(ii, act=act, n=n, A=A, Dn=Dn, Es=Es):
                              j = act[ii]
                              qlo, w = qrange(j)
                              E = Es.pop(ii)
                              nc.tensor.matmul(A[:, qlo:qlo + w], v_sb[j], E[:, 0:w],
                                               start=(ii == 0), stop=(ii == n - 1))
                              nc.tensor.matmul(Dn[:, qlo:qlo + w], ones_k, E[:, 0:w],
                                               start=(ii == 0), stop=(ii == n - 1))

                          LAG = 2
                          for ii in range(n + LAG):
                              if ii < n:
                                  emit_front(ii)
                              if ii >= LAG:
                                  emit_back(ii - LAG)

                          rec = p2r2.tile([1, 512], f32r, name="rec")
                          nc.vector.reciprocal(rec, Dn[0:1, :])
                          bsb = p2o.tile([128, 512], f32r, name="bsb")
                          nc.gpsimd.partition_broadcast(bsb, rec, 128)
                          nc.vector.tensor_mul(
                              att8[:, h, s * 512:(s + 1) * 512], A, bsb)

              # ---------------- Phase 3: output projection ------------------------------
              with ExitStack() as st3:
                  p3o = st3.enter_context(tc.tile_pool(name="p3o", bufs=4))
                  ps3 = st3.enter_context(tc.tile_pool(name="ps3", bufs=4, space="PSUM"))
                  for c in range(8):        # dout chunks of 512
                      wt = wt_next3
                      if c + 1 < 8:
                          wt_next3 = load_w3(c + 1)
                      for m in range(NKT):  # t tiles of 128
                          ps = ps3.tile([128, 512], f32, name="ps3")
                          for k in range(NHL):
                              nc.tensor.matmul(ps, att8[:, k, m * 128:(m + 1) * 128],
                                               wt[:, k, :],
                                               start=(k == 0), stop=(k == NHL - 1))
                          ot = p3o.tile([128, 512], f16, name="o3")
                          nc.vector.tensor_copy(ot, ps)
                          nc.scalar.dma_start(
                              out=out_d[m * 128:(m + 1) * 128, c * 512:(c + 1) * 512],
                              in_=ot)

    nc.compile()
    return nc


def _host_prep(x, wq, wk, wv, wo, freqs_cos, freqs_sin, mask):
    """Build per-core blob inputs + mask block classes."""
    x = np.asarray(x, np.float32)
    wq = np.asarray(wq, np.float32)
    wk = np.asarray(wk, np.float32)
    wv = np.asarray(wv, np.float32)
    wo = np.asarray(wo, np.float32)
    mask2 = np.asarray(mask, np.float32).reshape(S, S)
    maskt = np.ascontiguousarray(mask2.T)

    perm = np.concatenate(
        [hl * 128 + np.concatenate([np.arange(0, 128, 2), np.arange(1, 128, 2)])
         for hl in range(NHL)])
    cosw = np.ascontiguousarray(np.asarray(freqs_cos, np.float32).T).astype(np.float16)
    sinw = np.ascontiguousarray(np.asarray(freqs_sin, np.float32).T).astype(np.float16)
    nsinw = np.ascontiguousarray(-sinw)
    id128 = np.eye(128, dtype=np.float16)

    classes = [[0] * TSTRIPS for _ in range(NKT)]
    for j in range(NKT):
        for s in range(TSTRIPS):
            blk = maskt[j * 128:(j + 1) * 128, s * 512:(s + 1) * 512]
            if (blk <= NEG_THRESH).all():
                classes[j][s] = 0
            elif (blk == 0.0).all():
                classes[j][s] = 1
            else:
                classes[j][s] = 2

    # 4 multiplicative diagonal patterns, indexed by p = j - 4s
    mask4 = np.ones((4, 128, 512), np.float16)
    seen = [False] * 4
    for j in range(NKT):
        for s in range(TSTRIPS):
            if classes[j][s] != 2:
                continue
            p = j - 4 * s
            assert 0 <= p < 4, f"diagonal block offset {p} out of range"
            pat = (maskt[j * 128:(j + 1) * 128, s * 512:(s + 1) * 512]
                   > NEG_THRESH).astype(np.float16)
            if seen[p]:
                assert np.array_equal(mask4[p], pat), "inconsistent diag patterns"
            else:
                mask4[p] = pat
                seen[p] = True

    xts = [np.ascontiguousarray(x[b].T).astype(np.float16) for b in range(2)]
    sc = np.float32(1.0 / np.sqrt(HD))
    in_maps = []
    for core in range(8):
        b, tp = core // 4, core % 4
        sl = slice(tp * FSH, (tp + 1) * FSH)
        def pack_w(wt_DF):
            # [D, FSH] -> [NHL, 128p, KT, 128f] contiguous (SBUF tile layout)
            return np.ascontiguousarray(
                wt_DF.reshape(KT, 128, NHL, 128).transpose(2, 1, 0, 3)
            ).reshape(D, FSH)

        def pack_wo(wot_FD):
            # [FSH, D] -> [8c, 128p, NHL, 512f] contiguous
            return np.ascontiguousarray(
                wot_FD.reshape(NHL, 128, 8, 512).transpose(2, 1, 0, 3)
            ).reshape(FSH, D)

        parts = {
            "xt": xts[b],
            "wqt": pack_w((wq[sl][perm] * sc).T.astype(np.float16)),
            "wkt": pack_w(wk[sl][perm].T.astype(np.float16)),
            "wvt": pack_w(wv[sl].T.astype(np.float16)),
            "wot": pack_wo(wo[:, sl].T.astype(np.float16)),
            "cosw": cosw, "sinw": sinw, "nsinw": nsinw,
            "mask4": mask4.reshape(4 * 128, 512),
            "id128": id128,
        }
        blob = np.zeros(_BLOB_N, np.float16)
        for name, r, c in _LAYOUT:
            o = _OFFS[name]
            a = parts[name]
            assert a.shape == (r, c), (name, a.shape, (r, c))
            blob[o:o + r * c] = np.ascontiguousarray(a).reshape(-1)
        in_maps.append({"blob": blob})
    return in_maps, classes


def kernel(x, wq, wk, wv, wo, freqs_cos, freqs_sin, mask, start_pos=0,
           _trace=False):
    from concourse import bass_utils
    in_maps, classes = _host_prep(x, wq, wk, wv, wo, freqs_cos, freqs_sin, mask)
    key = str(classes)
    if key not in _cache:
        _cache[key] = _build(classes)
    nc = _cache[key]
    res = bass_utils.run_bass_kernel_spmd(nc, in_maps, core_ids=list(range(8)),
                                          trace=_trace)
    out = np.zeros((2, S, D), np.float32)
    for core in range(8):
        out[core // 4] += res.results[core]["out"].astype(np.float32)
    kernel.last_result = res
    return out


if __name__ == "__main__":
    # compile-only smoke test
    classes = [[2 if j * 128 <= s * 512 + 511 and j * 128 + 127 > s * 512 else
                (1 if j * 128 + 127 <= s * 512 else 0)
                for s in range(TSTRIPS)] for j in range(NKT)]
    import time
    t0 = time.time()
    nc = _build(classes)
    print(f"build+bacc-compile: {time.time()-t0:.1f}s")
    try:
        from concourse.timeline_sim import TimelineSim
        est = TimelineSim(nc, trace=False).simulate()
        print(f"TimelineSim per-core exec estimate: {est:.0f} ns")
    except Exception as e:
        print("TimelineSim unavailable:", e)
    if len(sys.argv) > 1 and sys.argv[1] == "neff":
        import tempfile
        from concourse import bass_utils
        t0 = time.time()
        with tempfile.TemporaryDirectory() as td:
            bass_utils.compile_bass_kernel(nc, td)
            print(f"walrus: {time.time()-t0:.1f}s COMPILED OK")

